# revision 12
# baseline (speedup 1.0000x reference)
"""LEConvMultiEdge Trainium2 kernel (8 NeuronCores, SPMD data-parallel).

Math (per batch b, dest node i, channel c):
  out = sigmoid(V@w1 + sum_l deg_l * (V@w2_l) - sum_l A_l @ (V@w3_l))
  deg_l[i] = sum_j A[b,i,j,l]

Device strategy: shard the 4096 (b,i) destination rows over 8 cores (512
each). The host pre-arranges each core's A shard as At[l, j, i] so the
contraction dim (j) lands on SBUF partitions with no on-chip transpose.
One PSUM accumulation chain of 64 matmuls (stationary = [U3_l | e_l]
68-wide, moving = At chunk [128 j, 512 i]) yields term3^T and all four
deg_l rows in a single bank. term1/term2 are tiny matmuls off V^T.
"""

import sys

if "/opt/trn_rl_repo" not in sys.path:
    sys.path.insert(0, "/opt/trn_rl_repo")

import numpy as np

B, N, F, C, L = 2, 2048, 64, 64, 4
P = 128
NCORES = 8
SH_PER_B = NCORES // B  # 4 shards per batch entry
IPC = N // SH_PER_B  # 512 dest rows per core
NJT = N // P  # 16 j-tiles
NCHUNK = L * NJT  # 64 contraction chunks
SW = C + L  # stationary width: 64 U3 cols + 4 deg one-hot cols

# bf16 storage for the A stream (2x less HBM traffic). U[0,1] entries in
# bf16 keep ~3 decimal digits; the 8192-term contraction stays well inside
# the harness tolerance (measured).
USE_BF16 = False

_NC_CACHE = {}


def _grp(bf16):
    return 8 if bf16 else 4  # chunks per At DMA -> 1 MiB transfers either way


def _build_nc(bf16=USE_BF16, reps=1):
    import concourse.bacc as bacc
    import concourse.bass as bass
    import concourse.mybir as mybir
    import concourse.tile as tile

    dt = mybir.dt.float32
    dta = mybir.dt.bfloat16 if bf16 else mybir.dt.float32
    GRP = _grp(bf16)
    NGRP = NCHUNK // GRP

    nc = bacc.Bacc("TRN2", debug=False, target_bir_lowering=False, num_devices=NCORES)

    At = nc.dram_tensor("At", [NGRP, P, GRP * IPC], dta, kind="ExternalInput")
    VT = nc.dram_tensor("VT", [F, N], dt, kind="ExternalInput")
    VTo = nc.dram_tensor("VTo", [F, IPC], dt, kind="ExternalInput")
    w1 = nc.dram_tensor("w1", [F, C], dt, kind="ExternalInput")
    w2 = nc.dram_tensor("w2", [L * F, C], dt, kind="ExternalInput")
    w3 = nc.dram_tensor("w3", [L * F, C], dt, kind="ExternalInput")
    out_d = nc.dram_tensor("out", [IPC, C], dt, kind="ExternalOutput")

    with tile.TileContext(nc) as tc:
        with (
            tc.tile_pool(name="const", bufs=1) as constp,
            tc.tile_pool(name="ats", bufs=3) as atp,
            tc.tile_pool(name="psum", bufs=1, space=bass.MemorySpace.PSUM) as psum,
            tc.tile_pool(name="psub", bufs=2, space=bass.MemorySpace.PSUM) as psub,
            tc.tile_pool(name="work", bufs=1) as work,
        ):
            # ---- load V^T and weights
            vt = constp.tile([F, N], dt)
            nc.sync.dma_start(vt[:], VT[:])
            vto = constp.tile([F, IPC], dt)
            nc.sync.dma_start(vto[:], VTo[:])
            w1s = constp.tile([F, C], dt)
            nc.sync.dma_start(w1s[:], w1[:])
            w2s = constp.tile([F, L * C], dt)
            w3s = constp.tile([F, L * C], dt)
            for l in range(L):
                nc.sync.dma_start(w2s[:, l * C : (l + 1) * C], w2[l * F : (l + 1) * F, :])
                nc.sync.dma_start(w3s[:, l * C : (l + 1) * C], w3[l * F : (l + 1) * F, :])

            # ---- build stationary U3' [128, NCHUNK*SW]:
            #   chunk q=(l*NJT+J): cols 0:C = (V @ w3_l)[j-tile J], col C+l = 1
            u3 = constp.tile([P, NCHUNK * SW], dta)
            u3r = u3[:].rearrange("p (q e) -> p q e", e=SW)
            nc.vector.memset(u3r[:, :, C : C + L], 0.0)
            for l in range(L):
                nc.vector.memset(u3r[:, l * NJT : (l + 1) * NJT, C + l], 1.0)
            for J in range(NJT):
                ub = psub.tile([P, L * C], dt)
                nc.tensor.matmul(
                    ub[:], vt[:, J * P : (J + 1) * P], w3s[:], start=True, stop=True
                )
                dst = u3[:].rearrange("p (l r) -> p l r", l=L)[
                    :, :, J * SW : J * SW + C
                ]
                src = ub[:].rearrange("p (l c) -> p l c", l=L)
                nc.vector.tensor_copy(dst, src)

            # ---- big contraction: 64 matmuls accumulating into one bank
            acc = psum.tile([SW, IPC], dt)
            for rep in range(reps):
                # reps>1 re-streams A for on-device timing; each rep restarts
                # the accumulation group, so the final contents stay correct.
                for g in range(NGRP):
                    at = atp.tile([P, GRP * IPC], dta)
                    nc.sync.dma_start(at[:], At[g])
                    for c4 in range(GRP):
                        q = g * GRP + c4
                        nc.tensor.matmul(
                            acc[:],
                            u3[:, q * SW : (q + 1) * SW],
                            at[:, c4 * IPC : (c4 + 1) * IPC],
                            start=(q == 0),
                            stop=(q == NCHUNK - 1),
                        )

            # ---- epilogue in [i, c] orientation (4 i-tiles of 128)
            # identity for TensorE transpose
            ident = constp.tile([P, P], dt)
            nc.vector.memset(ident[:], 1.0)
            nc.gpsimd.affine_select(
                ident[:],
                ident[:],
                [[1, P]],
                mybir.AluOpType.is_equal,
                0.0,
                base=0,
                channel_multiplier=-1,
            )
            accs_s = work.tile([SW, IPC], dt)
            nc.vector.tensor_copy(accs_s[:], acc[:])

            for it in range(N // P // SH_PER_B):  # 4 i-tiles
                trp = psub.tile([P, SW], dt, tag="trp")
                nc.tensor.transpose(
                    trp[:], accs_s[:, it * P : (it + 1) * P], ident[0:SW, 0:SW]
                )
                tr = work.tile([P, SW], dt, tag="tr")
                nc.vector.tensor_copy(tr[:], trp[:])
                # term1 and U2_l for this i-tile: stationary = VTo slice
                mm = psub.tile([P, (L + 1) * C], dt, tag="mm")
                lhs = vto[:, it * P : (it + 1) * P]
                nc.tensor.matmul(mm[:, 0:C], lhs, w1s[:], start=True, stop=True)
                for l in range(L):
                    nc.tensor.matmul(
                        mm[:, (l + 1) * C : (l + 2) * C],
                        lhs,
                        w2s[:, l * C : (l + 1) * C],
                        start=True,
                        stop=True,
                    )
                s = work.tile([P, C], dt, tag="s")
                # s = term1 - term3
                nc.vector.tensor_sub(s[:], mm[:, 0:C], tr[:, 0:C])
                for l in range(L):
                    tmp = work.tile([P, C], dt, tag="tmp")
                    nc.vector.tensor_scalar_mul(
                        tmp[:],
                        mm[:, (l + 1) * C : (l + 2) * C],
                        tr[:, C + l : C + l + 1],
                    )
                    nc.vector.tensor_add(s[:], s[:], tmp[:])
                o = work.tile([P, C], dt, tag="o")
                nc.scalar.activation(o[:], s[:], mybir.ActivationFunctionType.Sigmoid)
                nc.sync.dma_start(out_d[it * P : (it + 1) * P, :], o[:])

    nc.compile()
    return nc


def _get_nc(bf16=None):
    if bf16 is None:
        bf16 = USE_BF16
    key = ("nc", bf16)
    if key not in _NC_CACHE:
        _NC_CACHE[key] = _build_nc(bf16)
    return _NC_CACHE[key]


def _shard_inputs(V, A, w1, w2, w3, bf16=None):
    if bf16 is None:
        bf16 = USE_BF16
    GRP = _grp(bf16)
    NGRP = NCHUNK // GRP
    V = np.ascontiguousarray(np.asarray(V, dtype=np.float32))
    A = np.asarray(A, dtype=np.float32)
    w1 = np.ascontiguousarray(np.asarray(w1, dtype=np.float32))
    w2 = np.ascontiguousarray(np.asarray(w2, dtype=np.float32))
    w3 = np.ascontiguousarray(np.asarray(w3, dtype=np.float32))
    if bf16:
        import ml_dtypes

        dta_np = ml_dtypes.bfloat16
    else:
        dta_np = np.float32
    in_maps = []
    for k in range(NCORES):
        b, sshard = divmod(k, SH_PER_B)
        i0 = sshard * IPC
        Asl = A[b, i0 : i0 + IPC]  # (IPC, N, L)
        At3 = Asl.transpose(2, 1, 0)  # (L, N, IPC) = [l, j, i]
        Atg = (
            At3.reshape(NGRP, GRP, P, IPC)
            .transpose(0, 2, 1, 3)
            .reshape(NGRP, P, GRP * IPC)
        )
        in_maps.append(
            {
                "At": np.ascontiguousarray(Atg).astype(dta_np),
                "VT": np.ascontiguousarray(V[b].T),
                "VTo": np.ascontiguousarray(V[b, i0 : i0 + IPC].T),
                "w1": w1,
                "w2": w2,
                "w3": w3,
            }
        )
    return in_maps


LAST_EXEC_NS = None


def kernel(V, A, w1, w2, w3, _trace=False):
    global LAST_EXEC_NS
    from concourse.bass_utils import run_bass_kernel_spmd

    nc = _get_nc()
    in_maps = _shard_inputs(V, A, w1, w2, w3)
    res = run_bass_kernel_spmd(nc, in_maps, list(range(NCORES)), trace=_trace)
    LAST_EXEC_NS = res.exec_time_ns
    out = np.empty((B, N, C), dtype=np.float32)
    for k in range(NCORES):
        b, sshard = divmod(k, SH_PER_B)
        i0 = sshard * IPC
        out[b, i0 : i0 + IPC] = res.results[k]["out"]
    return out
